# revision 8
# baseline (speedup 1.0000x reference)
"""DD-RoPE kernel for 8x TRN2 NeuronCores.

Reference computation (B=4, T=4096, D=2048, P=256):
    deltas = einsum('btd,pd->btp', x, W) + b     # (B, T, P)
    angles = cumsum(deltas, axis=1)
    out = concat([x1*cos(a) - x2*sin(a), x2*cos(a) + x1*sin(a), x[..., 512:]], -1)

Sharding: 8 shards = 4 batches x 2 T-halves (2048 each). The cumsum carry
into the second T-half is a per-shard [256] vector computed on host in
float64 (sum of x rows @ W^T + 2048*b) and passed as the scan's initial
state, so there is no cross-core communication.

Per-core dataflow (all tensors in [feature-partition, time-free] layout):
    xt  [2048, 2048] = x_shard^T        (host pre-transposed)
    wt  [2048, 256]  = (W / 2pi)^T      (turns units)
    deltas^T [256, 2048] = wt^T @ xt + b/2pi     (PE matmul, fp32)
    angles^T = prefix-scan(deltas^T) + carry      (DVE tensor_tensor_scan)
    range-reduce in turns (magic-number rounding), sin/cos via ScalarE Sin
    rotated^T = [x1t*cos - x2t*sin ; x2t*cos + x1t*sin]   (DVE)
    outT [512, 2048] -> host transposes back; passthrough cols copied on host.
"""

import sys

if "/opt/trn_rl_repo" not in sys.path:
    sys.path.insert(0, "/opt/trn_rl_repo")

from contextlib import ExitStack

import numpy as np

import concourse.bacc as bacc
import concourse.bass as bass
import concourse.mybir as mybir
import concourse.tile as tile
from concourse.bass_utils import run_bass_kernel_spmd

F32 = mybir.dt.float32
ADD = mybir.AluOpType.add
SUB = mybir.AluOpType.subtract
IDENT = mybir.ActivationFunctionType.Identity
SIN = mybir.ActivationFunctionType.Sin

D = 2048          # input feature dim (contraction)
P = 256           # delta-pairs dim
ROT = 2 * P       # rotated columns (512)
TL = 2048         # time steps per shard
TB = 512          # time block (one PSUM bank at fp32)
KC = D // 128     # contraction chunks (16)
N_CORES = 8

MAGIC = 12582912.0          # 1.5 * 2**23: fp32 round-to-int magic constant
SCALE_2PI = 6.28310         # slightly < 2*pi so Sin args stay inside [-pi, pi]
COS_BIAS = 1.5707964        # ~pi/2 (fp32)
MM_DTYPE = F32              # matmul operand dtype


def build_program(tl: int = TL, mm_dtype=MM_DTYPE) -> bass.Bass:
    nt = tl // TB
    nc = bacc.Bacc("TRN2", target_bir_lowering=False, debug=False)

    xt = nc.dram_tensor("xt", [D, tl], F32, kind="ExternalInput").ap()
    wt = nc.dram_tensor("wt", [D, P], F32, kind="ExternalInput").ap()
    bv = nc.dram_tensor("bv", [1, P], F32, kind="ExternalInput").ap()
    cv = nc.dram_tensor("cv", [P, 1], F32, kind="ExternalInput").ap()
    outT = nc.dram_tensor("outT", [ROT, tl], F32, kind="ExternalOutput").ap()

    with tile.TileContext(nc) as tc, ExitStack() as ctx:
        const_pool = ctx.enter_context(tc.tile_pool(name="const", bufs=1))
        w_pool = ctx.enter_context(tc.tile_pool(name="w", bufs=1))
        x_pool = ctx.enter_context(tc.tile_pool(name="x", bufs=2))
        psum_pool = ctx.enter_context(tc.tile_pool(name="psum", bufs=4, space="PSUM"))
        ang_pool = ctx.enter_context(tc.tile_pool(name="ang", bufs=3))
        trig_pool = ctx.enter_context(tc.tile_pool(name="trig", bufs=2))
        rot_pool = ctx.enter_context(tc.tile_pool(name="rot", bufs=2))

        # Constants / weights
        wt_sb = []
        for d in range(KC):
            w_t = w_pool.tile([128, P], mm_dtype, tag=f"w{d}")
            nc.sync.dma_start(w_t[:], wt[d * 128:(d + 1) * 128, :])
            wt_sb.append(w_t)
        bv_sb = const_pool.tile([1, P], mm_dtype, tag="bv")
        nc.sync.dma_start(bv_sb[:], bv[:])
        cv_sb = const_pool.tile([128, 2], F32, tag="cv")
        nc.sync.dma_start(cv_sb[:, 0:1], cv[0:128, :])
        nc.sync.dma_start(cv_sb[:, 1:2], cv[128:256, :])
        ones_sb = const_pool.tile([1, TB], mm_dtype, tag="ones")
        nc.gpsimd.memset(ones_sb[:], 1.0)
        zeros_sb = const_pool.tile([128, TB], F32, tag="zeros")
        nc.gpsimd.memset(zeros_sb[:], 0.0)
        magic_sb = const_pool.tile([128, 1], F32, tag="magic")
        nc.gpsimd.memset(magic_sb[:], MAGIC)
        negq_sb = const_pool.tile([128, 1], F32, tag="negq")
        nc.gpsimd.memset(negq_sb[:], -0.25)
        cosb_sb = const_pool.tile([128, 1], F32, tag="cosb")
        nc.gpsimd.memset(cosb_sb[:], COS_BIAS)

        # Running scan-carry AP per p-half ([128, 1], fp32)
        prev_ang = [cv_sb[:, 0:1], cv_sb[:, 1:2]]

        for tb in range(nt):
            ts = slice(tb * TB, (tb + 1) * TB)
            # x^T tiles for this time block (d-chunk partitioned)
            xts = []
            for d in range(KC):
                x_t = x_pool.tile([128, TB], mm_dtype, tag=f"x{d}")
                nc.sync.dma_start(x_t[:], xt[d * 128:(d + 1) * 128, ts])
                xts.append(x_t)

            for h in range(2):
                ps = slice(h * 128, (h + 1) * 128)
                # deltas^T (+bias) for this (time block, p-half) in PSUM
                dp = psum_pool.tile([128, TB], F32, tag="dp")
                nc.tensor.matmul(dp[:], bv_sb[0:1, ps], ones_sb[:],
                                 start=True, stop=False)
                for d in range(KC):
                    nc.tensor.matmul(dp[:], wt_sb[d][:, ps], xts[d][:],
                                     start=False, stop=(d == KC - 1))

                # cumulative angle (turns) along time
                ang = ang_pool.tile([128, TB], F32, tag=f"ang{h}")
                nc.vector.tensor_tensor_scan(
                    ang[:], dp[:], zeros_sb[:], initial=prev_ang[h],
                    op0=ADD, op1=ADD)
                prev_ang[h] = ang[:, TB - 1:TB]

                # range reduction (turns): rs = y - round(y) in [-0.5, 0.5]
                a_s = trig_pool.tile([128, TB], F32, tag="a_s")
                nc.scalar.activation(a_s[:], ang[:], IDENT,
                                     bias=magic_sb[:], scale=-1.0)
                rs = trig_pool.tile([128, TB], F32, tag="rs")
                nc.vector.scalar_tensor_tensor(rs[:], a_s[:], MAGIC, ang[:],
                                               op0=SUB, op1=ADD)
                sin_t = trig_pool.tile([128, TB], F32, tag="sin")
                nc.scalar.activation(sin_t[:], rs[:], SIN, scale=SCALE_2PI)

                # rc = y - round(y + 0.25) in [-0.75, 0.25];
                # cos(2pi*y) = sin(2pi*rc + pi/2)
                b1 = trig_pool.tile([128, TB], F32, tag="b1")
                nc.scalar.activation(b1[:], ang[:], IDENT,
                                     bias=negq_sb[:], scale=-1.0)
                ac = trig_pool.tile([128, TB], F32, tag="ac")
                nc.scalar.activation(ac[:], b1[:], IDENT, bias=magic_sb[:])
                rc = trig_pool.tile([128, TB], F32, tag="rc")
                nc.vector.scalar_tensor_tensor(rc[:], ac[:], MAGIC, ang[:],
                                               op0=SUB, op1=ADD)
                cos_t = trig_pool.tile([128, TB], F32, tag="cos")
                nc.scalar.activation(cos_t[:], rc[:], SIN,
                                     scale=SCALE_2PI, bias=cosb_sb[:])

                # rotation: x1^T rows = d-chunk h, x2^T rows = d-chunk 2+h
                x1t, x2t = xts[h], xts[2 + h]
                t1 = rot_pool.tile([128, TB], F32, tag="t1")
                nc.vector.tensor_mul(t1[:], x1t[:], cos_t[:])
                t2 = rot_pool.tile([128, TB], F32, tag="t2")
                nc.vector.tensor_mul(t2[:], x2t[:], sin_t[:])
                o1 = rot_pool.tile([128, TB], F32, tag="o1")
                nc.vector.tensor_sub(o1[:], t1[:], t2[:])
                t3 = rot_pool.tile([128, TB], F32, tag="t3")
                nc.vector.tensor_mul(t3[:], x2t[:], cos_t[:])
                t4 = rot_pool.tile([128, TB], F32, tag="t4")
                nc.vector.tensor_mul(t4[:], x1t[:], sin_t[:])
                o2 = rot_pool.tile([128, TB], F32, tag="o2")
                nc.vector.tensor_add(o2[:], t3[:], t4[:])

                nc.sync.dma_start(outT[h * 128:(h + 1) * 128, ts], o1[:])
                nc.sync.dma_start(outT[P + h * 128:P + (h + 1) * 128, ts], o2[:])

    nc.compile()
    return nc


_NC_CACHE: dict = {}


def _get_nc():
    if "nc" not in _NC_CACHE:
        _NC_CACHE["nc"] = build_program()
    return _NC_CACHE["nc"]


def make_in_maps(x: np.ndarray, W: np.ndarray, b: np.ndarray):
    inv2pi = 1.0 / (2.0 * np.pi)
    Wt32 = (W.astype(np.float64) * inv2pi).astype(np.float32)      # [P, D]
    bt32 = (b.astype(np.float64) * inv2pi).astype(np.float32)      # [P]
    wt_in = np.ascontiguousarray(Wt32.T)                           # [D, P]
    bv_in = bt32[None, :]                                          # [1, P]

    # fp64 carry into the second T-half of each batch
    half = TL
    xsum = x[:, :half, :].sum(axis=1, dtype=np.float64)            # [B, D]
    carry = xsum @ Wt32.astype(np.float64).T + half * bt32.astype(np.float64)

    in_maps = []
    for c in range(N_CORES):
        bb, hh = c // 2, c % 2
        xt_in = np.ascontiguousarray(x[bb, hh * half:(hh + 1) * half, :].T)
        cvec = carry[bb] if hh else np.zeros(P)
        in_maps.append({
            "xt": xt_in,
            "wt": wt_in,
            "bv": bv_in,
            "cv": cvec.astype(np.float32)[:, None],
        })
    return in_maps


def assemble_output(x: np.ndarray, results) -> np.ndarray:
    B, T, Din = x.shape
    out = np.empty((B, T, Din), np.float32)
    out[:, :, ROT:] = x[:, :, ROT:]
    for c in range(N_CORES):
        bb, hh = c // 2, c % 2
        ot = results[c]["outT"]                                    # [512, TL]
        out[bb, hh * TL:(hh + 1) * TL, :ROT] = ot.T
    return out


def kernel(x: np.ndarray, W: np.ndarray, b: np.ndarray) -> np.ndarray:
    nc = _get_nc()
    in_maps = make_in_maps(x, W, b)
    res = run_bass_kernel_spmd(nc, in_maps, list(range(N_CORES)))
    return assemble_output(x, res.results)


# revision 20
# speedup vs baseline: 1.0292x; 1.0292x over previous
"""DD-RoPE kernel for 8x TRN2 NeuronCores.

Reference computation (B=4, T=4096, D=2048, P=256):
    deltas = einsum('btd,pd->btp', x, W) + b     # (B, T, P)
    angles = cumsum(deltas, axis=1)
    out = concat([x1*cos(a) - x2*sin(a), x2*cos(a) + x1*sin(a), x[..., 512:]], -1)

Sharding: 8 shards = 4 batches x 2 T-halves (2048 each). The cumsum carry
into the second T-half is a per-shard [256] vector computed on host in
float64 (sum of x rows @ W^T + 2048*b) and passed as the scan's initial
state, so there is no cross-core communication.

Per-core dataflow (all tensors in [feature-partition, time-free] layout):
    xt  [2048, 2048] = x_shard^T        (host pre-transposed, bf16 hi+lo pair)
    wt  [2048, 256]  = (W / 2pi)^T      (turns units, bf16 hi+lo pair)
    deltas^T = wh^T@xh + wl^T@xh + wh^T@xl + b   (PE, split-precision bf16:
               error ~2^-18 per product, needed because the cumsum amplifies
               per-delta error by sqrt(T))
    angles^T = prefix-scan(deltas^T) + carry      (DVE tensor_tensor_scan)
    range-reduce in turns (magic-number rounding), sin/cos via ScalarE Sin
    rotated^T = [x1t*cos - x2t*sin ; x2t*cos + x1t*sin]   (DVE, x = xh+xl)
    outT [512, 2048] -> host transposes back; passthrough cols copied on host.
"""

import sys

if "/opt/trn_rl_repo" not in sys.path:
    sys.path.insert(0, "/opt/trn_rl_repo")

from contextlib import ExitStack

import ml_dtypes
import numpy as np

import concourse.bacc as bacc
import concourse.bass as bass
import concourse.mybir as mybir
import concourse.tile as tile
from concourse.bass_utils import run_bass_kernel_spmd

F32 = mybir.dt.float32
BF16 = mybir.dt.bfloat16
ADD = mybir.AluOpType.add
SUB = mybir.AluOpType.subtract
IDENT = mybir.ActivationFunctionType.Identity
SIN = mybir.ActivationFunctionType.Sin

D = 2048          # input feature dim (contraction)
P = 256           # delta-pairs dim
ROT = 2 * P       # rotated columns (512)
TL = 2048         # time steps per shard
TB = 512          # time block (one PSUM bank at fp32)
KC = D // 128     # contraction chunks (16)
N_CORES = 8

MAGIC = 12582912.0          # 1.5 * 2**23: fp32 round-to-int magic constant
SCALE_2PI = 6.28310         # slightly < 2*pi so Sin args stay inside [-pi, pi]
COS_BIAS = 1.5707964        # ~pi/2 (fp32)
NP_BF16 = np.dtype(ml_dtypes.bfloat16)


def build_program(tl: int = TL) -> bass.Bass:
    nt = tl // TB
    nc = bacc.Bacc("TRN2", target_bir_lowering=False, debug=False)

    xh = nc.dram_tensor("xh", [D, tl], BF16, kind="ExternalInput").ap()
    xl = nc.dram_tensor("xl", [D, tl], BF16, kind="ExternalInput").ap()
    wh = nc.dram_tensor("wh", [D, P], BF16, kind="ExternalInput").ap()
    wl = nc.dram_tensor("wl", [D, P], BF16, kind="ExternalInput").ap()
    bv = nc.dram_tensor("bv", [1, 2 * P], BF16, kind="ExternalInput").ap()  # hi|lo
    cv = nc.dram_tensor("cv", [P, 1], F32, kind="ExternalInput").ap()
    outT = nc.dram_tensor("outT", [ROT, tl], F32, kind="ExternalOutput").ap()

    with tile.TileContext(nc) as tc, ExitStack() as ctx:
        const_pool = ctx.enter_context(tc.tile_pool(name="const", bufs=1))
        w_pool = ctx.enter_context(tc.tile_pool(name="w", bufs=1))
        x_pool = ctx.enter_context(tc.tile_pool(name="x", bufs=2))
        psum_pool = ctx.enter_context(tc.tile_pool(name="psum", bufs=4, space="PSUM"))
        ang_pool = ctx.enter_context(tc.tile_pool(name="ang", bufs=3))
        trig_pool = ctx.enter_context(tc.tile_pool(name="trig", bufs=2))
        rot_pool = ctx.enter_context(tc.tile_pool(name="rot", bufs=2))

        # Weights (stationary): bf16 hi/lo pairs per 128-chunk of D
        wh_sb, wl_sb = [], []
        for d in range(KC):
            w_t = w_pool.tile([128, P], BF16, tag=f"wh{d}")
            nc.sync.dma_start(w_t[:], wh[d * 128:(d + 1) * 128, :])
            wh_sb.append(w_t)
            w_t = w_pool.tile([128, P], BF16, tag=f"wl{d}")
            nc.sync.dma_start(w_t[:], wl[d * 128:(d + 1) * 128, :])
            wl_sb.append(w_t)
        bv_sb = const_pool.tile([1, 2 * P], BF16, tag="bv")
        nc.sync.dma_start(bv_sb[:], bv[:])
        cv_sb = const_pool.tile([128, 2], F32, tag="cv")
        nc.sync.dma_start(cv_sb[:, 0:1], cv[0:128, :])
        nc.sync.dma_start(cv_sb[:, 1:2], cv[128:256, :])
        ones_sb = const_pool.tile([1, TB], BF16, tag="ones")
        nc.gpsimd.memset(ones_sb[:], 1.0)
        zeros_sb = const_pool.tile([128, TB], F32, tag="zeros")
        nc.gpsimd.memset(zeros_sb[:], 0.0)
        magic_sb = const_pool.tile([128, 1], F32, tag="magic")
        nc.gpsimd.memset(magic_sb[:], MAGIC)
        negq_sb = const_pool.tile([128, 1], F32, tag="negq")
        nc.gpsimd.memset(negq_sb[:], -0.25)
        cosb_sb = const_pool.tile([128, 1], F32, tag="cosb")
        nc.gpsimd.memset(cosb_sb[:], COS_BIAS)

        # Running scan-carry AP per p-half ([128, 1], fp32)
        prev_ang = [cv_sb[:, 0:1], cv_sb[:, 1:2]]

        for tb in range(nt):
            ts = slice(tb * TB, (tb + 1) * TB)
            # x^T tiles for this time block (d-chunk partitioned, hi+lo)
            xhs, xls = [], []
            for d in range(KC):
                x_t = x_pool.tile([128, TB], BF16, tag=f"xh{d}")
                nc.sync.dma_start(x_t[:], xh[d * 128:(d + 1) * 128, ts])
                xhs.append(x_t)
                x_t = x_pool.tile([128, TB], BF16, tag=f"xl{d}")
                nc.sync.dma_start(x_t[:], xl[d * 128:(d + 1) * 128, ts])
                xls.append(x_t)

            for h in range(2):
                ps = slice(h * 128, (h + 1) * 128)
                # deltas^T (+bias) in PSUM: wh@xh + wl@xh + wh@xl + b_hi + b_lo
                dp = psum_pool.tile([128, TB], F32, tag="dp")
                nc.tensor.matmul(dp[:], bv_sb[0:1, ps], ones_sb[:],
                                 start=True, stop=False)
                nc.tensor.matmul(dp[:], bv_sb[0:1, P + h * 128:P + (h + 1) * 128],
                                 ones_sb[:], start=False, stop=False)
                for d in range(KC):
                    nc.tensor.matmul(dp[:], wh_sb[d][:, ps], xhs[d][:],
                                     start=False, stop=False)
                    nc.tensor.matmul(dp[:], wl_sb[d][:, ps], xhs[d][:],
                                     start=False, stop=False)
                    nc.tensor.matmul(dp[:], wh_sb[d][:, ps], xls[d][:],
                                     start=False, stop=(d == KC - 1))

                # cumulative angle (turns) along time
                ang = ang_pool.tile([128, TB], F32, tag=f"ang{h}")
                nc.vector.tensor_tensor_scan(
                    ang[:], dp[:], zeros_sb[:], initial=prev_ang[h],
                    op0=ADD, op1=ADD)
                prev_ang[h] = ang[:, TB - 1:TB]

                # range reduction (turns): rs = y - round(y) in [-0.5, 0.5]
                a_s = trig_pool.tile([128, TB], F32, tag="a_s")
                nc.scalar.activation(a_s[:], ang[:], IDENT,
                                     bias=magic_sb[:], scale=-1.0)
                rs = trig_pool.tile([128, TB], F32, tag="rs")
                nc.vector.scalar_tensor_tensor(rs[:], a_s[:], MAGIC, ang[:],
                                               op0=SUB, op1=ADD)
                sin_t = trig_pool.tile([128, TB], F32, tag="sin")
                nc.scalar.activation(sin_t[:], rs[:], SIN, scale=SCALE_2PI)

                # rc = y - round(y + 0.25) in [-0.75, 0.25];
                # cos(2pi*y) = sin(2pi*rc + pi/2)
                b1 = trig_pool.tile([128, TB], F32, tag="b1")
                nc.scalar.activation(b1[:], ang[:], IDENT,
                                     bias=negq_sb[:], scale=-1.0)
                ac = trig_pool.tile([128, TB], F32, tag="ac")
                nc.scalar.activation(ac[:], b1[:], IDENT, bias=magic_sb[:])
                rc = trig_pool.tile([128, TB], F32, tag="rc")
                nc.vector.scalar_tensor_tensor(rc[:], ac[:], MAGIC, ang[:],
                                               op0=SUB, op1=ADD)
                cos_t = trig_pool.tile([128, TB], F32, tag="cos")
                nc.scalar.activation(cos_t[:], rc[:], SIN,
                                     scale=SCALE_2PI, bias=cosb_sb[:])

                # rotation: x1^T rows = d-chunk h, x2^T rows = d-chunk 2+h;
                # reconstruct x = xh + xl in fp32 on DVE
                x1t = rot_pool.tile([128, TB], F32, tag="x1t")
                nc.vector.tensor_add(x1t[:], xhs[h][:], xls[h][:])
                x2t = rot_pool.tile([128, TB], F32, tag="x2t")
                nc.vector.tensor_add(x2t[:], xhs[2 + h][:], xls[2 + h][:])
                t1 = rot_pool.tile([128, TB], F32, tag="t1")
                nc.vector.tensor_mul(t1[:], x1t[:], cos_t[:])
                t2 = rot_pool.tile([128, TB], F32, tag="t2")
                nc.vector.tensor_mul(t2[:], x2t[:], sin_t[:])
                o1 = rot_pool.tile([128, TB], F32, tag="o1")
                nc.vector.tensor_sub(o1[:], t1[:], t2[:])
                t3 = rot_pool.tile([128, TB], F32, tag="t3")
                nc.vector.tensor_mul(t3[:], x2t[:], cos_t[:])
                t4 = rot_pool.tile([128, TB], F32, tag="t4")
                nc.vector.tensor_mul(t4[:], x1t[:], sin_t[:])
                o2 = rot_pool.tile([128, TB], F32, tag="o2")
                nc.vector.tensor_add(o2[:], t3[:], t4[:])

                nc.sync.dma_start(outT[h * 128:(h + 1) * 128, ts], o1[:])
                nc.sync.dma_start(outT[P + h * 128:P + (h + 1) * 128, ts], o2[:])

    nc.compile()
    return nc


_NC_CACHE: dict = {}


def _get_nc():
    if "nc" not in _NC_CACHE:
        _NC_CACHE["nc"] = build_program()
    return _NC_CACHE["nc"]


def _split_bf16(a64: np.ndarray):
    """Split a float64 array into bf16 hi + bf16 lo (lo = bf16(a - hi))."""
    hi = a64.astype(NP_BF16)
    lo = (a64 - hi.astype(np.float64)).astype(NP_BF16)
    return np.ascontiguousarray(hi), np.ascontiguousarray(lo)


def make_in_maps(x: np.ndarray, W: np.ndarray, b: np.ndarray):
    inv2pi = 1.0 / (2.0 * np.pi)
    Wt = W.astype(np.float64) * inv2pi                             # [P, D]
    bt = b.astype(np.float64) * inv2pi                             # [P]
    wh_in, wl_in = _split_bf16(Wt.T)                               # [D, P]
    bh = bt.astype(NP_BF16)
    bl = (bt - bh.astype(np.float64)).astype(NP_BF16)
    bv_in = np.ascontiguousarray(
        np.concatenate([bh, bl])[None, :])                         # [1, 2P]

    # fp64 carry into the second T-half of each batch, using the same
    # effective (hi+lo) weight values the device uses
    half = TL
    w_eff = wh_in.astype(np.float64) + wl_in.astype(np.float64)    # [D, P]
    b_eff = bh.astype(np.float64) + bl.astype(np.float64)
    xsum = x[:, :half, :].sum(axis=1, dtype=np.float64)            # [B, D]
    carry = xsum @ w_eff + half * b_eff                            # [B, P]

    in_maps = []
    for c in range(N_CORES):
        bb, hh = c // 2, c % 2
        xt64 = x[bb, hh * half:(hh + 1) * half, :].T.astype(np.float64)
        xh_in, xl_in = _split_bf16(xt64)
        cvec = carry[bb] if hh else np.zeros(P)
        in_maps.append({
            "xh": xh_in,
            "xl": xl_in,
            "wh": wh_in,
            "wl": wl_in,
            "bv": bv_in,
            "cv": cvec.astype(np.float32)[:, None],
        })
    return in_maps


def assemble_output(x: np.ndarray, results) -> np.ndarray:
    B, T, Din = x.shape
    out = np.empty((B, T, Din), np.float32)
    out[:, :, ROT:] = x[:, :, ROT:]
    for c in range(N_CORES):
        bb, hh = c // 2, c % 2
        ot = results[c]["outT"]                                    # [512, TL]
        out[bb, hh * TL:(hh + 1) * TL, :ROT] = ot.T
    return out


def kernel(x: np.ndarray, W: np.ndarray, b: np.ndarray) -> np.ndarray:
    nc = _get_nc()
    in_maps = make_in_maps(x, W, b)
    res = run_bass_kernel_spmd(nc, in_maps, list(range(N_CORES)))
    return assemble_output(x, res.results)


# revision 23
# speedup vs baseline: 1.2451x; 1.2097x over previous
"""DD-RoPE kernel for 8x TRN2 NeuronCores.

Reference computation (B=4, T=4096, D=2048, P=256):
    deltas = einsum('btd,pd->btp', x, W) + b     # (B, T, P)
    angles = cumsum(deltas, axis=1)
    out = concat([x1*cos(a) - x2*sin(a), x2*cos(a) + x1*sin(a), x[..., 512:]], -1)

Sharding: 8 shards = 4 batches x 2 T-halves (2048 each). The cumsum carry
into the second T-half is a per-shard [256] vector computed on host in
float64 (sum of x rows @ W^T + 2048*b) and passed as the scan's initial
state, so there is no cross-core communication.

Per-core dataflow (all tensors in [feature-partition, time-free] layout):
    x^T in bf16 hi+lo pairs, host pre-tiled so every DMA is one dense block
    deltas^T = wh^T@xh + wl^T@xh + wh^T@xl        (PE, split-precision bf16:
               error ~2^-18 per product, needed because the cumsum amplifies
               per-delta error by sqrt(T))
    angles^T = prefix-scan(deltas^T, +bias) + carry   (DVE tensor_tensor_scan,
               bias folded in via the scan's second operand)
    range-reduce in turns (magic-number rounding), sin/cos via ScalarE Sin
    rotated^T = [x1t*cos - x2t*sin ; x2t*cos + x1t*sin]   (DVE, x = xh+xl)
    outT tiled [oi, h, tb, 128, 512] -> host reassembles; passthrough cols
    copied on host.
"""

import sys

if "/opt/trn_rl_repo" not in sys.path:
    sys.path.insert(0, "/opt/trn_rl_repo")

from contextlib import ExitStack

import ml_dtypes
import numpy as np

import concourse.bacc as bacc
import concourse.bass as bass
import concourse.mybir as mybir
import concourse.tile as tile
from concourse.bass_utils import run_bass_kernel_spmd

F32 = mybir.dt.float32
BF16 = mybir.dt.bfloat16
ADD = mybir.AluOpType.add
SUB = mybir.AluOpType.subtract
IDENT = mybir.ActivationFunctionType.Identity
SIN = mybir.ActivationFunctionType.Sin

D = 2048          # input feature dim (contraction)
P = 256           # delta-pairs dim
ROT = 2 * P       # rotated columns (512)
TL = 2048         # time steps per shard
TB = 512          # time block (one PSUM bank at fp32)
NT = TL // TB     # time blocks (4)
KC = D // 128     # contraction chunks (16)
N_CORES = 8

MAGIC = 12582912.0          # 1.5 * 2**23: fp32 round-to-int magic constant
SCALE_2PI = 6.28310         # slightly < 2*pi so Sin args stay inside [-pi, pi]
COS_BIAS = 1.5707964        # ~pi/2 (fp32)
NP_BF16 = np.dtype(ml_dtypes.bfloat16)


def build_program(tl: int = TL) -> bass.Bass:
    nt = tl // TB
    nc = bacc.Bacc("TRN2", target_bir_lowering=False, debug=False)

    # Host-pre-tiled inputs: every DMA below reads one dense DRAM block.
    # xhl row block (d, tb): [128, hi(TB) | lo(TB)] bf16
    xhl = nc.dram_tensor("xhl", [KC * nt * 128, 2 * TB], BF16,
                         kind="ExternalInput").ap()
    # whl row block d: [128, wh(P) | wl(P)] bf16
    whl = nc.dram_tensor("whl", [KC * 128, 2 * P], BF16,
                         kind="ExternalInput").ap()
    bv = nc.dram_tensor("bv", [1, 2 * P], BF16, kind="ExternalInput").ap()
    cv = nc.dram_tensor("cv", [P, 1], F32, kind="ExternalInput").ap()
    # out row block (oi, h, tb): [128, TB] f32
    outT = nc.dram_tensor("outT", [4 * nt * 128, TB], F32,
                          kind="ExternalOutput").ap()

    with tile.TileContext(nc) as tc, ExitStack() as ctx:
        const_pool = ctx.enter_context(tc.tile_pool(name="const", bufs=1))
        w_pool = ctx.enter_context(tc.tile_pool(name="w", bufs=1))
        x_pool = ctx.enter_context(tc.tile_pool(name="x", bufs=2))
        psum_pool = ctx.enter_context(tc.tile_pool(name="psum", bufs=4, space="PSUM"))
        ang_pool = ctx.enter_context(tc.tile_pool(name="ang", bufs=3))
        trig_pool = ctx.enter_context(tc.tile_pool(name="trig", bufs=2))
        rot_pool = ctx.enter_context(tc.tile_pool(name="rot", bufs=2))

        # Weights (stationary): one dense DMA per d-chunk
        w_sb = []
        for d in range(KC):
            w_t = w_pool.tile([128, 2 * P], BF16, tag=f"w{d}")
            nc.sync.dma_start(w_t[:], whl[d * 128:(d + 1) * 128, :])
            w_sb.append(w_t)
        cv_sb = const_pool.tile([128, 2], F32, tag="cv")
        nc.sync.dma_start(cv_sb[:, 0:1], cv[0:128, :])
        nc.sync.dma_start(cv_sb[:, 1:2], cv[128:256, :])
        bv_sb = const_pool.tile([1, 2 * P], BF16, tag="bv")
        nc.sync.dma_start(bv_sb[:], bv[:])
        ones_sb = const_pool.tile([1, TB], BF16, tag="ones")
        nc.gpsimd.memset(ones_sb[:], 1.0)
        zeros_sb = const_pool.tile([128, TB], F32, tag="zeros")
        nc.gpsimd.memset(zeros_sb[:], 0.0)
        magic_sb = const_pool.tile([128, 1], F32, tag="magic")
        nc.gpsimd.memset(magic_sb[:], MAGIC)
        negq_sb = const_pool.tile([128, 1], F32, tag="negq")
        nc.gpsimd.memset(negq_sb[:], -0.25)
        cosb_sb = const_pool.tile([128, 1], F32, tag="cosb")
        nc.gpsimd.memset(cosb_sb[:], COS_BIAS)

        # Running scan-carry AP per p-half ([128, 1], fp32)
        prev_ang = [cv_sb[:, 0:1], cv_sb[:, 1:2]]

        for tb in range(nt):
            # x^T tiles: one dense DMA per (d, tb), [128, hi|lo]
            xts = []
            for d in range(KC):
                x_t = x_pool.tile([128, 2 * TB], BF16, tag=f"x{d}")
                r0 = (d * nt + tb) * 128
                nc.sync.dma_start(x_t[:], xhl[r0:r0 + 128, :])
                xts.append(x_t)

            for h in range(2):
                hs = slice(h * 128, (h + 1) * 128)          # wh columns
                ls = slice(P + h * 128, P + (h + 1) * 128)  # wl columns
                # deltas^T (+bias) in PSUM: b_hi + b_lo + wh@xh + wh@xl + wl@xh
                dp = psum_pool.tile([128, TB], F32, tag="dp")
                nc.tensor.matmul(dp[:], bv_sb[0:1, h * 128:(h + 1) * 128],
                                 ones_sb[:], start=True, stop=False)
                nc.tensor.matmul(dp[:], bv_sb[0:1, P + h * 128:P + (h + 1) * 128],
                                 ones_sb[:], start=False, stop=False)
                for d in range(KC):
                    xhi = xts[d][:, 0:TB]
                    xlo = xts[d][:, TB:2 * TB]
                    nc.tensor.matmul(dp[:], w_sb[d][:, hs], xhi,
                                     start=False, stop=False)
                    nc.tensor.matmul(dp[:], w_sb[d][:, hs], xlo,
                                     start=False, stop=False)
                    nc.tensor.matmul(dp[:], w_sb[d][:, ls], xhi,
                                     start=False, stop=(d == KC - 1))

                # cumulative angle (turns) along time; bias folded in via op1
                ang = ang_pool.tile([128, TB], F32, tag=f"ang{h}")
                nc.vector.tensor_tensor_scan(
                    ang[:], dp[:], zeros_sb[:], initial=prev_ang[h],
                    op0=ADD, op1=ADD)
                prev_ang[h] = ang[:, TB - 1:TB]

                # range reduction (turns): rs = y - round(y) in [-0.5, 0.5]
                a_s = trig_pool.tile([128, TB], F32, tag="a_s")
                nc.scalar.activation(a_s[:], ang[:], IDENT,
                                     bias=magic_sb[:], scale=-1.0)
                rs = trig_pool.tile([128, TB], F32, tag="rs")
                nc.vector.scalar_tensor_tensor(rs[:], a_s[:], MAGIC, ang[:],
                                               op0=SUB, op1=ADD)
                sin_t = trig_pool.tile([128, TB], F32, tag="sin")
                nc.scalar.activation(sin_t[:], rs[:], SIN, scale=SCALE_2PI)

                # rc = y - round(y + 0.25) in [-0.75, 0.25];
                # cos(2pi*y) = sin(2pi*rc + pi/2)
                b1 = trig_pool.tile([128, TB], F32, tag="b1")
                nc.scalar.activation(b1[:], ang[:], IDENT,
                                     bias=negq_sb[:], scale=-1.0)
                ac = trig_pool.tile([128, TB], F32, tag="ac")
                nc.scalar.activation(ac[:], b1[:], IDENT, bias=magic_sb[:])
                rc = trig_pool.tile([128, TB], F32, tag="rc")
                nc.vector.scalar_tensor_tensor(rc[:], ac[:], MAGIC, ang[:],
                                               op0=SUB, op1=ADD)
                cos_t = trig_pool.tile([128, TB], F32, tag="cos")
                nc.scalar.activation(cos_t[:], rc[:], SIN,
                                     scale=SCALE_2PI, bias=cosb_sb[:])

                # rotation: x1^T rows = d-chunk h, x2^T rows = d-chunk 2+h;
                # reconstruct x = xh + xl in fp32 on DVE
                x1t = rot_pool.tile([128, TB], F32, tag="x1t")
                nc.vector.tensor_add(x1t[:], xts[h][:, 0:TB],
                                     xts[h][:, TB:2 * TB])
                x2t = rot_pool.tile([128, TB], F32, tag="x2t")
                nc.vector.tensor_add(x2t[:], xts[2 + h][:, 0:TB],
                                     xts[2 + h][:, TB:2 * TB])
                t1 = rot_pool.tile([128, TB], F32, tag="t1")
                nc.vector.tensor_mul(t1[:], x1t[:], cos_t[:])
                t2 = rot_pool.tile([128, TB], F32, tag="t2")
                nc.vector.tensor_mul(t2[:], x2t[:], sin_t[:])
                o1 = rot_pool.tile([128, TB], F32, tag="o1")
                nc.vector.tensor_sub(o1[:], t1[:], t2[:])
                t3 = rot_pool.tile([128, TB], F32, tag="t3")
                nc.vector.tensor_mul(t3[:], x2t[:], cos_t[:])
                t4 = rot_pool.tile([128, TB], F32, tag="t4")
                nc.vector.tensor_mul(t4[:], x1t[:], sin_t[:])
                o2 = rot_pool.tile([128, TB], F32, tag="o2")
                nc.vector.tensor_add(o2[:], t3[:], t4[:])

                r1 = ((0 * 2 + h) * nt + tb) * 128
                r2 = ((1 * 2 + h) * nt + tb) * 128
                nc.sync.dma_start(outT[r1:r1 + 128, :], o1[:])
                nc.sync.dma_start(outT[r2:r2 + 128, :], o2[:])

    nc.compile()
    return nc


_NC_CACHE: dict = {}


def _get_nc():
    if "nc" not in _NC_CACHE:
        _NC_CACHE["nc"] = build_program()
    return _NC_CACHE["nc"]


def _split_bf16_f32(a32: np.ndarray):
    """Split float32 -> bf16 hi + bf16 lo (lo = bf16(a - hi), exact residual)."""
    hi = a32.astype(NP_BF16)
    lo = (a32 - hi.astype(np.float32)).astype(NP_BF16)
    return hi, lo


def _tile_x(xt32: np.ndarray, nt: int) -> np.ndarray:
    """[D, tl] f32 -> [KC*nt*128, 2*TB] bf16, block (d, tb) = [128, hi|lo]."""
    tl = xt32.shape[1]
    hi, lo = _split_bf16_f32(xt32)
    hi = hi.reshape(KC, 128, nt, TB).transpose(0, 2, 1, 3)
    lo = lo.reshape(KC, 128, nt, TB).transpose(0, 2, 1, 3)
    out = np.empty((KC, nt, 128, 2 * TB), dtype=NP_BF16)
    out[..., :TB] = hi
    out[..., TB:] = lo
    return np.ascontiguousarray(out.reshape(KC * nt * 128, 2 * TB))


def make_in_maps(x: np.ndarray, W: np.ndarray, b: np.ndarray):
    inv2pi = 1.0 / (2.0 * np.pi)
    Wt = W.astype(np.float64) * inv2pi                             # [P, D]
    bt = b.astype(np.float64) * inv2pi                             # [P]
    wh = Wt.T.astype(NP_BF16)                                      # [D, P]
    wl = (Wt.T - wh.astype(np.float64)).astype(NP_BF16)
    whl_in = np.empty((KC, 128, 2 * P), dtype=NP_BF16)
    whl_in[..., :P] = wh.reshape(KC, 128, P)
    whl_in[..., P:] = wl.reshape(KC, 128, P)
    whl_in = np.ascontiguousarray(whl_in.reshape(KC * 128, 2 * P))
    bh = bt.astype(NP_BF16)
    bl = (bt - bh.astype(np.float64)).astype(NP_BF16)
    bv_in = np.ascontiguousarray(np.concatenate([bh, bl])[None, :])  # [1, 2P]

    # fp64 carry into the second T-half of each batch, using the same
    # effective (hi+lo) weight values the device uses
    half = TL
    w_eff = wh.astype(np.float64) + wl.astype(np.float64)          # [D, P]
    b_eff = bh.astype(np.float64) + bl.astype(np.float64)
    xsum = x[:, :half, :].sum(axis=1, dtype=np.float64)            # [B, D]
    carry = xsum @ w_eff + half * b_eff                            # [B, P]

    in_maps = []
    for c in range(N_CORES):
        bb, hh = c // 2, c % 2
        xt32 = np.ascontiguousarray(x[bb, hh * half:(hh + 1) * half, :].T)
        cvec = carry[bb] if hh else np.zeros(P)
        in_maps.append({
            "xhl": _tile_x(xt32, NT),
            "whl": whl_in,
            "bv": bv_in,
            "cv": cvec.astype(np.float32)[:, None],
        })
    return in_maps


def assemble_output(x: np.ndarray, results) -> np.ndarray:
    B, T, Din = x.shape
    out = np.empty((B, T, Din), np.float32)
    out[:, :, ROT:] = x[:, :, ROT:]
    for c in range(N_CORES):
        bb, hh = c // 2, c % 2
        r = results[c]["outT"].reshape(2, 2, NT, 128, TB)
        # [oi, h, tb, pp, u] -> [t_local(tb,u), p(oi,h,pp)]
        blk = r.transpose(2, 4, 0, 1, 3).reshape(TL, ROT)
        out[bb, hh * TL:(hh + 1) * TL, :ROT] = blk
    return out


def kernel(x: np.ndarray, W: np.ndarray, b: np.ndarray) -> np.ndarray:
    nc = _get_nc()
    in_maps = make_in_maps(x, W, b)
    res = run_bass_kernel_spmd(nc, in_maps, list(range(N_CORES)))
    return assemble_output(x, res.results)


# revision 24
# speedup vs baseline: 1.4157x; 1.1370x over previous
"""DD-RoPE kernel for 8x TRN2 NeuronCores.

Reference computation (B=4, T=4096, D=2048, P=256):
    deltas = einsum('btd,pd->btp', x, W) + b     # (B, T, P)
    angles = cumsum(deltas, axis=1)
    out = concat([x1*cos(a) - x2*sin(a), x2*cos(a) + x1*sin(a), x[..., 512:]], -1)

Sharding: 8 shards = 4 batches x 2 T-halves (2048 each), data-parallel.
The cumsum is handled with host-computed fp64 "block bases": the exact
cumulative angle at every 128-step boundary (one [256, 16] vector set per
shard, computed from 128-step block sums of x in one pass). Each on-device
prefix scan then only spans 128 steps, so per-delta rounding error from the
reduced-precision matmul amplifies by at most sqrt(128), and there is no
cross-core (or even cross-block) dependency at all.

Per-core dataflow (all tensors in [feature-partition, time-free] layout):
    xf  [2048, 2048] fp16 = fp16(x_shard^T), host pre-tiled per (d, tb)
    wf  [2048, 512] fp16 = [W_hi | W_lo * 2^12] in turns units
    xs = xf * 2^-12 (DVE, fp16)  -- scale trick keeps W_lo out of fp16
                                    subnormal range; powers of 2 are exact
    deltas^T = wh^T@xf + wlo_s^T@xs + b_hi + b_lo   (PE fp16, 2 passes,
               + 2 bf16 rank-1 bias matmuls, accumulated in fp32 PSUM)
    angles^T = per-128 prefix scans of deltas^T, initial = host base
    range-reduce in turns (magic-number rounding), sin/cos via ScalarE Sin
    rotated^T = [x1*cos - x2*sin ; x2*cos + x1*sin]   (DVE, x = fp16 x)
    outT tiled [oi, h, tb, 128, 512] -> host reassembles; passthrough cols
    copied on host.
"""

import sys

if "/opt/trn_rl_repo" not in sys.path:
    sys.path.insert(0, "/opt/trn_rl_repo")

from contextlib import ExitStack

import ml_dtypes
import numpy as np

import concourse.bacc as bacc
import concourse.bass as bass
import concourse.mybir as mybir
import concourse.tile as tile
from concourse.bass_utils import run_bass_kernel_spmd

F32 = mybir.dt.float32
F16 = mybir.dt.float16
BF16 = mybir.dt.bfloat16
ADD = mybir.AluOpType.add
SUB = mybir.AluOpType.subtract
IDENT = mybir.ActivationFunctionType.Identity
SIN = mybir.ActivationFunctionType.Sin

D = 2048          # input feature dim (contraction)
P = 256           # delta-pairs dim
ROT = 2 * P       # rotated columns (512)
TL = 2048         # time steps per shard
TB = 512          # time block (one PSUM bank at fp32)
SB = 128          # scan block (base injection granularity)
NT = TL // TB     # time blocks per shard (4)
NBK = TL // SB    # scan blocks per shard (16)
KC = D // 128     # contraction chunks (16)
N_CORES = 8

MAGIC = 12582912.0          # 1.5 * 2**23: fp32 round-to-int magic constant
SCALE_2PI = 6.28310         # slightly < 2*pi so Sin args stay inside [-pi, pi]
COS_BIAS = 1.5707964        # ~pi/2 (fp32)
LO_SCALE = 4096.0           # 2^12: W_lo pre-scale (exact power of 2)
NP_BF16 = np.dtype(ml_dtypes.bfloat16)


def build_program(tl: int = TL) -> bass.Bass:
    nt = tl // TB
    nbk = tl // SB
    nc = bacc.Bacc("TRN2", target_bir_lowering=False, debug=False)

    # Host-pre-tiled inputs: every DMA below reads one dense DRAM block.
    xf = nc.dram_tensor("xf", [KC * nt * 128, TB], F16,
                        kind="ExternalInput").ap()
    # wf row block d: [128, wh(P) | wlo_scaled(P)] fp16
    wf = nc.dram_tensor("wf", [KC * 128, 2 * P], F16,
                        kind="ExternalInput").ap()
    bv = nc.dram_tensor("bv", [1, 2 * P], BF16, kind="ExternalInput").ap()
    # per-128-block angle bases (turns), [P, nbk] fp32
    bs = nc.dram_tensor("bs", [P, nbk], F32, kind="ExternalInput").ap()
    # out row block (oi, h, tb): [128, TB] f32
    outT = nc.dram_tensor("outT", [4 * nt * 128, TB], F32,
                          kind="ExternalOutput").ap()

    with tile.TileContext(nc) as tc, ExitStack() as ctx:
        const_pool = ctx.enter_context(tc.tile_pool(name="const", bufs=1))
        w_pool = ctx.enter_context(tc.tile_pool(name="w", bufs=1))
        x_pool = ctx.enter_context(tc.tile_pool(name="x", bufs=2))
        xs_pool = ctx.enter_context(tc.tile_pool(name="xs", bufs=2))
        psum_pool = ctx.enter_context(tc.tile_pool(name="psum", bufs=4, space="PSUM"))
        ang_pool = ctx.enter_context(tc.tile_pool(name="ang", bufs=2))
        trig_pool = ctx.enter_context(tc.tile_pool(name="trig", bufs=2))
        rot_pool = ctx.enter_context(tc.tile_pool(name="rot", bufs=2))

        # Weights (stationary): one dense DMA per d-chunk
        w_sb = []
        for d in range(KC):
            w_t = w_pool.tile([128, 2 * P], F16, tag=f"w{d}")
            nc.sync.dma_start(w_t[:], wf[d * 128:(d + 1) * 128, :])
            w_sb.append(w_t)
        bs_sb = const_pool.tile([128, 2 * nbk], F32, tag="bs")
        nc.sync.dma_start(bs_sb[:, 0:nbk], bs[0:128, :])
        nc.sync.dma_start(bs_sb[:, nbk:2 * nbk], bs[128:256, :])
        bv_sb = const_pool.tile([1, 2 * P], BF16, tag="bv")
        nc.sync.dma_start(bv_sb[:], bv[:])
        ones_sb = const_pool.tile([1, TB], BF16, tag="ones")
        nc.gpsimd.memset(ones_sb[:], 1.0)
        zeros_sb = const_pool.tile([128, TB], F32, tag="zeros")
        nc.gpsimd.memset(zeros_sb[:], 0.0)
        magic_sb = const_pool.tile([128, 1], F32, tag="magic")
        nc.gpsimd.memset(magic_sb[:], MAGIC)
        negq_sb = const_pool.tile([128, 1], F32, tag="negq")
        nc.gpsimd.memset(negq_sb[:], -0.25)
        cosb_sb = const_pool.tile([128, 1], F32, tag="cosb")
        nc.gpsimd.memset(cosb_sb[:], COS_BIAS)

        for tb in range(nt):
            # x^T tiles (fp16) + derived scaled copies for the W_lo term
            xts, xss = [], []
            for d in range(KC):
                x_t = x_pool.tile([128, TB], F16, tag=f"x{d}")
                r0 = (d * nt + tb) * 128
                nc.sync.dma_start(x_t[:], xf[r0:r0 + 128, :])
                xts.append(x_t)
                s_t = xs_pool.tile([128, TB], F16, tag=f"xs{d}")
                nc.vector.tensor_scalar_mul(s_t[:], x_t[:], 1.0 / LO_SCALE)
                xss.append(s_t)

            for h in range(2):
                hs = slice(h * 128, (h + 1) * 128)          # wh columns
                ls = slice(P + h * 128, P + (h + 1) * 128)  # wlo columns
                # deltas^T (+bias) in PSUM: b_hi + b_lo + wh@xf + wlo_s@xs
                dp = psum_pool.tile([128, TB], F32, tag="dp")
                nc.tensor.matmul(dp[:], bv_sb[0:1, h * 128:(h + 1) * 128],
                                 ones_sb[:], start=True, stop=False)
                nc.tensor.matmul(dp[:], bv_sb[0:1, P + h * 128:P + (h + 1) * 128],
                                 ones_sb[:], start=False, stop=False)
                for d in range(KC):
                    nc.tensor.matmul(dp[:], w_sb[d][:, hs], xts[d][:],
                                     start=False, stop=False)
                    nc.tensor.matmul(dp[:], w_sb[d][:, ls], xss[d][:],
                                     start=False, stop=(d == KC - 1))

                # cumulative angle (turns): independent per-128 scans with
                # host-computed initial bases
                ang = ang_pool.tile([128, TB], F32, tag=f"ang{h}")
                for k in range(TB // SB):
                    kb = tb * (TB // SB) + k
                    cs = slice(k * SB, (k + 1) * SB)
                    nc.vector.tensor_tensor_scan(
                        ang[:, cs], dp[:, cs], zeros_sb[:, 0:SB],
                        initial=bs_sb[:, h * nbk + kb:h * nbk + kb + 1],
                        op0=ADD, op1=ADD)

                # range reduction (turns): rs = y - round(y) in [-0.5, 0.5]
                a_s = trig_pool.tile([128, TB], F32, tag="a_s")
                nc.scalar.activation(a_s[:], ang[:], IDENT,
                                     bias=magic_sb[:], scale=-1.0)
                rs = trig_pool.tile([128, TB], F32, tag="rs")
                nc.vector.scalar_tensor_tensor(rs[:], a_s[:], MAGIC, ang[:],
                                               op0=SUB, op1=ADD)
                sin_t = trig_pool.tile([128, TB], F32, tag="sin")
                nc.scalar.activation(sin_t[:], rs[:], SIN, scale=SCALE_2PI)

                # rc = y - round(y + 0.25) in [-0.75, 0.25];
                # cos(2pi*y) = sin(2pi*rc + pi/2)
                b1 = trig_pool.tile([128, TB], F32, tag="b1")
                nc.scalar.activation(b1[:], ang[:], IDENT,
                                     bias=negq_sb[:], scale=-1.0)
                ac = trig_pool.tile([128, TB], F32, tag="ac")
                nc.scalar.activation(ac[:], b1[:], IDENT, bias=magic_sb[:])
                rc = trig_pool.tile([128, TB], F32, tag="rc")
                nc.vector.scalar_tensor_tensor(rc[:], ac[:], MAGIC, ang[:],
                                               op0=SUB, op1=ADD)
                cos_t = trig_pool.tile([128, TB], F32, tag="cos")
                nc.scalar.activation(cos_t[:], rc[:], SIN,
                                     scale=SCALE_2PI, bias=cosb_sb[:])

                # rotation: x1^T rows = d-chunk h, x2^T rows = d-chunk 2+h
                t1 = rot_pool.tile([128, TB], F32, tag="t1")
                nc.vector.tensor_mul(t1[:], xts[h][:], cos_t[:])
                t2 = rot_pool.tile([128, TB], F32, tag="t2")
                nc.vector.tensor_mul(t2[:], xts[2 + h][:], sin_t[:])
                o1 = rot_pool.tile([128, TB], F32, tag="o1")
                nc.vector.tensor_sub(o1[:], t1[:], t2[:])
                t3 = rot_pool.tile([128, TB], F32, tag="t3")
                nc.vector.tensor_mul(t3[:], xts[2 + h][:], cos_t[:])
                t4 = rot_pool.tile([128, TB], F32, tag="t4")
                nc.vector.tensor_mul(t4[:], xts[h][:], sin_t[:])
                o2 = rot_pool.tile([128, TB], F32, tag="o2")
                nc.vector.tensor_add(o2[:], t3[:], t4[:])

                r1 = ((0 * 2 + h) * nt + tb) * 128
                r2 = ((1 * 2 + h) * nt + tb) * 128
                nc.sync.dma_start(outT[r1:r1 + 128, :], o1[:])
                nc.sync.dma_start(outT[r2:r2 + 128, :], o2[:])

    nc.compile()
    return nc


_NC_CACHE: dict = {}


def _get_nc():
    if "nc" not in _NC_CACHE:
        _NC_CACHE["nc"] = build_program()
    return _NC_CACHE["nc"]


def _tile_rows(a: np.ndarray, nt: int, tb: int) -> np.ndarray:
    """[D, tl] -> [KC*nt*128, tb]: row block (d, t_block) = [128, tb]."""
    tl = a.shape[1]
    out = a.reshape(KC, 128, tl // tb, tb).transpose(0, 2, 1, 3)
    return np.ascontiguousarray(out.reshape(KC * (tl // tb) * 128, tb))


def prepare_weights(W: np.ndarray, b: np.ndarray):
    inv2pi = 1.0 / (2.0 * np.pi)
    Wt = W.astype(np.float64).T * inv2pi                           # [D, P]
    bt = b.astype(np.float64) * inv2pi                             # [P]
    whf = Wt.astype(np.float16)
    wlo_s = ((Wt - whf.astype(np.float64)) * LO_SCALE).astype(np.float16)
    wf_in = np.empty((KC, 128, 2 * P), dtype=np.float16)
    wf_in[..., :P] = whf.reshape(KC, 128, P)
    wf_in[..., P:] = wlo_s.reshape(KC, 128, P)
    wf_in = np.ascontiguousarray(wf_in.reshape(KC * 128, 2 * P))
    bh = bt.astype(NP_BF16)
    bl = (bt - bh.astype(np.float64)).astype(NP_BF16)
    bv_in = np.ascontiguousarray(np.concatenate([bh, bl])[None, :])
    # device-effective weights/bias for the host base computation
    w_eff = whf.astype(np.float64) + wlo_s.astype(np.float64) / LO_SCALE
    b_eff = bh.astype(np.float64) + bl.astype(np.float64)
    return wf_in, bv_in, w_eff, b_eff


def make_in_maps(x: np.ndarray, W: np.ndarray, b: np.ndarray):
    B = x.shape[0]
    wf_in, bv_in, w_eff, b_eff = prepare_weights(W, b)

    # fp64 cumulative angle at every 128-step boundary, per batch (in turns):
    # one pass of 128-block sums over x, then a small [32, D] @ [D, P] matmul
    T = x.shape[1]
    nblk = T // SB                                                  # 32
    xblk = x.reshape(B, nblk, SB, D).sum(axis=2, dtype=np.float64)  # [B, 32, D]
    dblk = xblk @ w_eff + SB * b_eff                                # [B, 32, P]
    bases = np.zeros((B, nblk, P))
    np.cumsum(dblk[:, :-1], axis=1, out=bases[:, 1:])               # exclusive

    in_maps = []
    for c in range(N_CORES):
        bb, hh = c // 2, c % 2
        xt16 = x[bb, hh * TL:(hh + 1) * TL, :].T.astype(np.float16)
        bs_in = bases[bb, hh * NBK:(hh + 1) * NBK].T                # [P, NBK]
        in_maps.append({
            "xf": _tile_rows(xt16, NT, TB),
            "wf": wf_in,
            "bv": bv_in,
            "bs": np.ascontiguousarray(bs_in.astype(np.float32)),
        })
    return in_maps


def assemble_output(x: np.ndarray, results) -> np.ndarray:
    B, T, Din = x.shape
    out = np.empty((B, T, Din), np.float32)
    out[:, :, ROT:] = x[:, :, ROT:]
    for c in range(N_CORES):
        bb, hh = c // 2, c % 2
        r = results[c]["outT"].reshape(2, 2, NT, 128, TB)
        # [oi, h, tb, pp, u] -> [t_local(tb,u), p(oi,h,pp)]
        blk = r.transpose(2, 4, 0, 1, 3).reshape(TL, ROT)
        out[bb, hh * TL:(hh + 1) * TL, :ROT] = blk
    return out


def kernel(x: np.ndarray, W: np.ndarray, b: np.ndarray) -> np.ndarray:
    nc = _get_nc()
    in_maps = make_in_maps(x, W, b)
    res = run_bass_kernel_spmd(nc, in_maps, list(range(N_CORES)))
    return assemble_output(x, res.results)
